# revision 2
# baseline (speedup 1.0000x reference)
"""TRN2 Bass kernel for nn_Convolution_2d: 3x3 same-padding conv2d.

X (32,128,64,64) f32  *  W (256,128,3,3)  + bias (256,)  ->  (32,256,64,64)

Strategy: data-parallel over batch across 8 NeuronCores (4 images/core),
with 1D Winograd F(2,3) along the y (height) axis to cut TensorE work 1.5x
vs direct convolution (12 instead of 18 N=512 matmuls per pair of output
rows). The x axis stays direct (3 kx taps PSUM-accumulated via shifted
views); the y-axis input transform

    V_i(c, a, x) = sum_mu BT[i,mu] Xpad(c, 2a+mu, x)
    U_i(o, c, kx) = sum_ky G[i,ky] W(o, c, ky, kx)

is computed on the HOST (exact fp32, cast to fp16), so the device only runs

    M_i[o, a, x] = sum_kx U_i[:,:,kx].T @ V_i[:, a, x+kx]      (TensorE)
    out(2a+0) = M_0 + M_1 + M_2 + bias                          (ScalarE+DVE)
    out(2a+1) = M_1 - M_2 - M_3 + bias

The output transform is split so ScalarE does all PSUM reads (4 activation
copies per chunk, bias fused into s1) and DVE runs purely on fp16 SBUF
tiles at the 2x perf mode. All device data is fp16 (error ~5e-4 measured,
vs the 2e-2 gate); the host upcasts the fp16 output to fp32.
"""
import numpy as np
from contextlib import ExitStack

import jax
import concourse.bass as bass
import concourse.tile as tile
from concourse import bacc, mybir
from concourse.bass2jax import (
    _bass_exec_p,
    install_neuronx_cc_hook,
    partition_id_tensor,
)
from jax.sharding import Mesh, PartitionSpec
from jax.experimental.shard_map import shard_map

N_CORES = 8
B, CIN, H, W = 32, 128, 64, 64
COUT = 256
KH = KW = 3
PAD = 1
HP, WP = H + 2 * PAD, W + 2 * PAD   # 66, 66
BC = B // N_CORES                   # images per core = 4
NA = H // 2                         # y tile-pairs per image = 32
AQ = 8                              # tile-pairs per chunk (N = AQ*W = 512)
NQ = NA // AQ                       # chunks per (image, m) = 4
M_TILES = COUT // 128               # 2

f32 = mybir.dt.float32
f16 = mybir.dt.float16

# F(2,3) transform matrices (host side, exact)
_BT = np.array([[1, 0, -1, 0],
                [0, 1, 1, 0],
                [0, -1, 1, 0],
                [0, 1, 0, -1]], np.float32)
_G = np.array([[1, 0, 0],
               [.5, .5, .5],
               [.5, -.5, .5],
               [0, 0, 1]], np.float32)


def _build_module():
    nc = bacc.Bacc("TRN2", target_bir_lowering=False, debug=False,
                   num_devices=N_CORES)
    Vp = nc.declare_dram_parameter("Vp", [CIN, 4, BC, NA, WP], f16,
                                   isOutput=False)
    Ut = nc.declare_dram_parameter("Ut", [CIN, 4 * KW, COUT], f16,
                                   isOutput=False)
    bias2 = nc.declare_dram_parameter("bias2", [128, M_TILES], f32,
                                      isOutput=False)
    # out[o, b, p, a, x]: p=0 -> row 2a, p=1 -> row 2a+1 (host interleaves)
    out = nc.declare_dram_parameter("out", [COUT, BC, 2, NA, W], f16,
                                    isOutput=True)

    add = mybir.AluOpType.add
    sub = mybir.AluOpType.subtract
    ident = mybir.ActivationFunctionType.Identity

    with ExitStack() as ctx:
        tc = ctx.enter_context(tile.TileContext(nc))
        const = ctx.enter_context(tc.tile_pool(name="const", bufs=1))
        xpool = ctx.enter_context(tc.tile_pool(name="x", bufs=1))
        opool = ctx.enter_context(tc.tile_pool(name="o", bufs=16))
        psum = ctx.enter_context(tc.tile_pool(name="psum", bufs=8,
                                              space="PSUM"))

        v_sb = xpool.tile([CIN, 4, BC, NA, WP], f16)
        u_sb = const.tile([CIN, 4 * KW, COUT], f16)
        b_sb = const.tile([128, M_TILES], f32)

        # startup order: V rows the first chunks need, then weights, then rest
        for i in range(4):
            nc.sync.dma_start(v_sb[:, i, 0, 0:16], Vp[:, i, 0, 0:16])
        nc.sync.dma_start(u_sb[:], Ut[:])
        nc.sync.dma_start(b_sb[:], bias2[:])
        for i in range(4):
            nc.sync.dma_start(v_sb[:, i, 0, 16:NA], Vp[:, i, 0, 16:NA])
        for b in range(1, BC):
            for i in range(4):
                nc.sync.dma_start(v_sb[:, i, b], Vp[:, i, b])

        # warm the PE clock gate (HAM) during the initial DMA wait
        warm_f = const.tile([128, 128], f32)
        nc.vector.memset(warm_f[:], 0.0)
        warm_x = const.tile([128, 128], f16)
        nc.vector.tensor_copy(warm_x[:], warm_f[:])
        wps = psum.tile([128, 64], f32, tag="ps", name="warm_ps")
        for _ in range(20):
            nc.tensor.matmul(wps[:], warm_x[:], warm_x[:, 0:64],
                             start=True, stop=True)

        for b in range(BC):
            for m in range(M_TILES):
                mo = slice(m * 128, (m + 1) * 128)
                for q in range(NQ):
                    a0 = q * AQ
                    pss = [psum.tile([128, AQ * W], f32, tag="ps",
                                     name=f"ps{i}") for i in range(4)]
                    for i in range(4):
                        for kx in range(KW):
                            nc.tensor.matmul(
                                pss[i][:],
                                u_sb[:, i * KW + kx, mo],
                                v_sb[:, i, b, a0:a0 + AQ, kx:kx + W],
                                start=(kx == 0), stop=(kx == KW - 1))
                    s0 = opool.tile([128, AQ * W], f16, name="s0")
                    s1 = opool.tile([128, AQ * W], f16, name="s1")
                    s2 = opool.tile([128, AQ * W], f16, name="s2")
                    s3 = opool.tile([128, AQ * W], f16, name="s3")
                    nc.scalar.copy(s0[:], pss[0][:])
                    nc.scalar.activation(s1[:], pss[1][:], ident,
                                         bias=b_sb[:, m:m + 1])
                    nc.scalar.copy(s2[:], pss[2][:])
                    nc.scalar.copy(s3[:], pss[3][:])
                    tp = opool.tile([128, AQ * W], f16, name="tp")
                    tm = opool.tile([128, AQ * W], f16, name="tm")
                    o0 = opool.tile([128, AQ * W], f16, name="o0")
                    o1 = opool.tile([128, AQ * W], f16, name="o1")
                    nc.vector.tensor_tensor(tp[:], s1[:], s2[:], add)
                    nc.vector.tensor_tensor(tm[:], s1[:], s2[:], sub)
                    nc.vector.tensor_tensor(o0[:], tp[:], s0[:], add)
                    nc.vector.tensor_tensor(o1[:], tm[:], s3[:], sub)
                    nc.gpsimd.dma_start(out[mo, b, 0, a0:a0 + AQ], o0[:])
                    nc.gpsimd.dma_start(out[mo, b, 1, a0:a0 + AQ], o1[:])

    nc.compile()
    return nc


_CACHE = {}


def _get_runner():
    if "run" in _CACHE:
        return _CACHE["run"]

    install_neuronx_cc_hook()
    nc = _build_module()

    partition_name = nc.partition_id_tensor.name if nc.partition_id_tensor else None
    in_names, out_names, out_avals = [], [], []
    for alloc in nc.m.functions[0].allocations:
        if not isinstance(alloc, mybir.MemoryLocationSet):
            continue
        name = alloc.memorylocations[0].name
        if alloc.kind == "ExternalInput":
            if name != partition_name:
                in_names.append(name)
        elif alloc.kind == "ExternalOutput":
            out_names.append(name)
            out_avals.append(jax.core.ShapedArray(
                tuple(alloc.tensor_shape), mybir.dt.np(alloc.dtype)))
    n_params = len(in_names)
    all_in_names = list(in_names) + list(out_names)
    if partition_name is not None:
        all_in_names.append(partition_name)
    donate = tuple(range(n_params, n_params + len(out_names)))

    def _body(*args):
        operands = list(args)
        if partition_name is not None:
            operands.append(partition_id_tensor())
        return tuple(_bass_exec_p.bind(
            *operands,
            out_avals=tuple(out_avals),
            in_names=tuple(all_in_names),
            out_names=tuple(out_names),
            lowering_input_output_aliases=(),
            sim_require_finite=True,
            sim_require_nnan=True,
            nc=nc,
        ))

    devices = jax.devices()[:N_CORES]
    mesh = Mesh(np.asarray(devices), ("core",))
    n_io = n_params + len(out_names)
    jitted = jax.jit(
        shard_map(_body, mesh=mesh,
                  in_specs=(PartitionSpec("core"),) * n_io,
                  out_specs=(PartitionSpec("core"),) * len(out_names),
                  check_rep=False),
        donate_argnums=donate,
        keep_unused=True,
    )

    def run(per_core_inputs):
        concat_in = [
            np.concatenate([per_core_inputs[c][name] for c in range(N_CORES)], axis=0)
            for name in in_names
        ]
        concat_zeros = [
            np.zeros((N_CORES * a.shape[0], *a.shape[1:]), a.dtype) for a in out_avals
        ]
        out_arrs = jitted(*concat_in, *concat_zeros)
        jax.block_until_ready(out_arrs)
        return [
            {name: np.asarray(out_arrs[i]).reshape(N_CORES, *out_avals[i].shape)[c]
             for i, name in enumerate(out_names)}
            for c in range(N_CORES)
        ]

    _CACHE["run"] = run
    return run


def _prepare_inputs(X, weights, biases):
    Xpad = np.pad(X, ((0, 0), (0, 0), (PAD, PAD), (PAD, PAD)))  # (B,C,66,66)
    # V[i][b,c,a,x] = sum_mu BT[i,mu] Xpad[b,c,2a+mu,x]
    # strided view (B,C,NA,4,WP): rows 2a+mu
    sv = np.lib.stride_tricks.as_strided(
        Xpad,
        shape=(B, CIN, NA, 4, WP),
        strides=(Xpad.strides[0], Xpad.strides[1], 2 * Xpad.strides[2],
                 Xpad.strides[2], Xpad.strides[3]),
    )
    V = np.einsum('im,bcamx->bicax', _BT, sv).astype(np.float16)
    # U[i][o,c,kx] -> Ut[c, i*3+kx, o]
    U = np.einsum('ik,ockx->ciox', _G, weights)          # (C,4,O,KW)
    Ut = np.ascontiguousarray(
        U.transpose(0, 1, 3, 2).reshape(CIN, 4 * KW, COUT)).astype(np.float16)
    bias2 = np.ascontiguousarray(biases.reshape(M_TILES, 128).T)
    per_core = []
    for c in range(N_CORES):
        Vc = np.ascontiguousarray(
            V[c * BC:(c + 1) * BC].transpose(2, 1, 0, 3, 4))  # [C,4,BC,NA,WP]
        per_core.append({
            "Vp": Vc,
            "Ut": Ut,
            "bias2": bias2,
        })
    return per_core


def kernel(X, weights, biases):
    X = np.asarray(X, dtype=np.float32)
    weights = np.asarray(weights, dtype=np.float32)
    biases = np.asarray(biases, dtype=np.float32)

    run = _get_runner()
    results = run(_prepare_inputs(X, weights, biases))

    out = np.empty((B, COUT, H, W), dtype=np.float32)
    for c in range(N_CORES):
        o = results[c]["out"].astype(np.float32)   # [COUT, BC, 2, NA, W]
        o = o.transpose(1, 0, 2, 3, 4)             # [BC, COUT, 2, NA, W]
        out[c * BC:(c + 1) * BC, :, 0::2] = o[:, :, 0]
        out[c * BC:(c + 1) * BC, :, 1::2] = o[:, :, 1]
    return out
